# revision 6
# baseline (speedup 1.0000x reference)
"""Trainium2 Bass kernel for MockBitNetLayer:

    scale = mean(|W|, axis=1)            # [O, 1]
    y = x @ (sign(W) * scale).T + bias   # [T, O]

Column-parallel over 8 NeuronCores: each core owns an O/8 = 2048-row shard
of W and bias, reads the full x, and produces y[:, shard] in natural [T, O]
orientation; host concatenates along the feature axis.

v3 design (the v2 baseline traced at ~3.1ms; PE busy only 70%, HAM cold,
all input DMA serialized on one ~190GB/s queue):
  - MMs are stationary-reuse-first: the stationary operand is an x.T block
    [128k, 128t] reused across 4 o-chunk matmuls (LDWEIGHTS 1:4), with 8
    PSUM banks ping-ponging between accumulation and eviction.  Keeps the
    PE issuing back-to-back so the HAM clock-gate stays warm.
  - x.T per 128-token slab via a 2-hop pipeline: SWDGE cast-DMA (f32->fp16,
    DRAM->DRAM, ~240GB/s measured, no SBUF staging) then XBAR DMA-transpose
    into SBUF (~258GB/s measured).  The SWDGE ring carries ONLY x casts;
    XBARs ride sync/SP; evictions ride scalar/ACT.
  - W prep (~67MB of DMA, the startup critical path) runs on the two HWDGE
    queues: f32 tiles in (SP/ACT alternating), fused sign->fp16 on the
    scalar engine, exact f32 |W| scale reduce on the vector engine, fp16
    spill, then XBAR transpose straight into the resident swt
    [128, 32, 2048] o-chunk by o-chunk.
  - Startup runs CHUNK-MAJOR: as each of the 4 swt o-chunks lands, the PE
    sweeps the first 6 (SBUF-resident) slabs over just that chunk, so the
    PE has work the whole time W streams in.  Steady state is slab-major.
  - scale/bias applied at PSUM eviction on the otherwise-idle vector
    engine as free-dim row vectors (partition-broadcast fp16, per-chunk so
    evictions only gate on their own chunk's scale).

Error budget: x,W in fp16 (~2e-4 output rel err), scale fp16 (~5e-4) --
far under the 2e-2 gate.
"""

import os
import sys

for _p in ("/opt/trn_rl_repo", "/root/.axon_site/_ro/trn_rl_repo"):
    if os.path.isdir(_p) and _p not in sys.path:
        sys.path.insert(0, _p)

import numpy as np

import concourse.bacc as bacc
import concourse.mybir as mybir
import concourse.tile as tile
from concourse.bass import ds
from concourse.bass_utils import run_bass_kernel_spmd

P = 128
N_CORES = 8

# Full problem shapes (hardcoded per spec).
T_FULL = 8192
K_FULL = 4096
O_FULL = 16384


def build_kernel_body_v3(tc, x, w, b, y, T, K, O, TCH=128, PHASE_SLABS=6):
    """Emit the per-core program.

    x: [T, K] f32 (replicated)   w: [O, K] f32 (shard)
    b: [O]    f32 (shard)        y: [T, O] f32 out (shard of y, feature axis)
    """
    nc = tc.nc
    f32 = mybir.dt.float32
    f16 = mybir.dt.float16

    KT = K // P          # contraction tiles (32)
    OC = O // 512        # o-chunks = psum width (4)
    OT = O // P          # o-tiles (16)
    OTC = OT // OC       # o-tiles per o-chunk (4)
    NS = T // TCH        # token slabs (64)
    KS = K // 512        # 512-col k groups (8)
    PH = min(PHASE_SLABS, NS)

    with (
        tc.tile_pool(name="const", bufs=1) as const_pool,
        tc.tile_pool(name="swt", bufs=1) as swt_pool,
        tc.tile_pool(name="wstage", bufs=4) as wstage_pool,
        tc.tile_pool(name="swstage", bufs=2) as swstage_pool,
        tc.tile_pool(name="xt", bufs=PHASE_SLABS) as xt_pool,
        tc.tile_pool(name="out", bufs=2) as out_pool,
        tc.tile_pool(name="psum_mm", bufs=8, space="PSUM") as psum_mm,
        tc.tile_pool(name="dram", bufs=1, space="DRAM") as dram_pool,
        tc.tile_pool(name="dram_x", bufs=8, space="DRAM") as dram_x_pool,
    ):
        signw = dram_pool.tile([O, K], f16, name="signw")
        swt = swt_pool.tile([P, KT, O], f16)
        scale_sb = const_pool.tile([P, OT], f32)
        partials = const_pool.tile([P, KS], f32)
        scale_dram = dram_pool.tile([O], f32, name="scale_dram")
        scale_row16 = const_pool.tile([1, O], f16, name="scale_row16")
        scale_bc = const_pool.tile([P, O], f16, name="scale_bc")

        xh_tiles = {}
        xt_tiles = {}

        def issue_x_cast(s):
            xh = dram_x_pool.tile([TCH, K], f16, tag="xh", name=f"xh{s % 8}")
            nc.gpsimd.dma_start(xh, x[ds(s * TCH, TCH), :])
            xh_tiles[s] = xh

        def issue_xt(s):
            xt = xt_pool.tile([P, KT, TCH], f16, tag="xt", name=f"xt{s % PHASE_SLABS}")
            for g in range(KS):
                nc.sync.dma_start_transpose(
                    xt[:, ds(4 * g, 4), :], xh_tiles[s][:, ds(512 * g, 512)]
                )
            xt_tiles[s] = xt

        def mm_group(s, oc, interleaved_sibling=None):
            """One accumulation group (slab s x o-chunk oc) + eviction."""
            psum = psum_mm.tile([P, 512], f32, tag="acc", name=f"acc{oc}")
            xt = xt_tiles[s]
            for k in range(KT):
                nc.tensor.matmul(
                    psum,
                    lhsT=xt[:, k, ds(0, P)],
                    rhs=swt[:, k, ds(512 * oc, 512)],
                    start=(k == 0),
                    stop=(k == KT - 1),
                )
            evict(s, oc, psum)

        def evict(s, oc, psum):
            o_sb = out_pool.tile([P, 512], f32, tag="osb", name=f"o{oc}")
            nc.vector.tensor_mul(o_sb, psum, scale_bc[:, ds(512 * oc, 512)])
            nc.vector.tensor_add(o_sb, o_sb, bias_bc[:, ds(512 * oc, 512)])
            nc.scalar.dma_start(y[ds(s * TCH, P), ds(512 * oc, 512)], o_sb)

        # ---- W prep, o-chunk by o-chunk; x casts for phase slabs early ----
        for s in range(PH):
            issue_x_cast(s)
        bias_row16 = const_pool.tile([1, O], f16, name="bias_row16")
        nc.gpsimd.dma_start(bias_row16, b.rearrange("(one o) -> one o", one=1))
        bias_bc = const_pool.tile([P, O], f16, name="bias_bc")
        nc.gpsimd.partition_broadcast(bias_bc, bias_row16)

        for c in range(OC):
            for ot in range(OTC * c, OTC * (c + 1)):
                for g in range(KS):
                    wst = wstage_pool.tile(
                        [P, 512], f32, tag="wst", name=f"wst{g % 4}"
                    )
                    eng_ld = nc.sync if g % 2 == 0 else nc.scalar
                    eng_ld.dma_start(wst, w[ds(P * ot, P), ds(512 * g, 512)])
                    nc.vector.tensor_reduce(
                        out=partials[:, ds(g, 1)], in_=wst,
                        axis=mybir.AxisListType.X, op=mybir.AluOpType.add,
                        apply_absolute_value=True,
                    )
                    sw16 = swstage_pool.tile(
                        [P, 512], f16, tag="sw16", name=f"sw16{g % 2}"
                    )
                    nc.scalar.sign(sw16, wst)
                    eng_st = nc.scalar if g % 2 == 0 else nc.sync
                    eng_st.dma_start(
                        signw[ds(P * ot, P), ds(512 * g, 512)], sw16
                    )
                stot = const_pool.tile([P, 1], f32, tag="stot", name="stot")
                nc.vector.tensor_reduce(
                    out=stot, in_=partials, axis=mybir.AxisListType.X,
                    op=mybir.AluOpType.add,
                )
                nc.scalar.mul(scale_sb[:, ds(ot, 1)], stot, 1.0 / K)
            # XBAR this chunk straight into resident swt
            for g in range(KS):
                nc.sync.dma_start_transpose(
                    swt[:, ds(4 * g, 4), ds(512 * c, 512)],
                    signw[ds(512 * c, 512), ds(512 * g, 512)],
                )
            # per-chunk scale broadcast (DRAM roundtrip + cast + bcast)
            nc.sync.dma_start(
                scale_dram[ds(512 * c, 512)].rearrange("(j p) -> p j", p=P),
                scale_sb[:, ds(OTC * c, OTC)],
            )
            nc.gpsimd.dma_start(
                scale_row16[:, ds(512 * c, 512)],
                scale_dram[ds(512 * c, 512)].rearrange("(one o) -> one o", one=1),
            )
            nc.gpsimd.partition_broadcast(
                scale_bc[:, ds(512 * c, 512)], scale_row16[:, ds(512 * c, 512)]
            )

        # ---- startup: chunk-major phases over the first PH slabs ----
        for s in range(PH):
            issue_xt(s)
        nxt = PH
        for oc in range(OC):
            for s in range(PH):
                mm_group(s, oc)
                if nxt < min(NS, PH + 4 * (oc + 1)):
                    issue_x_cast(nxt)
                    nxt += 1

        # ---- steady state: slab-major, o-chunks interleaved ----
        for s in range(PH, NS):
            while nxt < min(NS, s + 6):
                issue_x_cast(nxt)
                nxt += 1
            issue_xt(s)
            psums = [
                psum_mm.tile([P, 512], f32, tag="acc", name=f"acc{oc}")
                for oc in range(OC)
            ]
            xt = xt_tiles[s]
            for k in range(KT):
                for oc in range(OC):
                    nc.tensor.matmul(
                        psums[oc],
                        lhsT=xt[:, k, ds(0, P)],
                        rhs=swt[:, k, ds(512 * oc, 512)],
                        start=(k == 0),
                        stop=(k == KT - 1),
                    )
            for oc in range(OC):
                evict(s, oc, psums[oc])


def build_bass(T=T_FULL, K=K_FULL, O=O_FULL // N_CORES, TCH=128):
    nc = bacc.Bacc(trn_type="TRN2")
    f32 = mybir.dt.float32
    x = nc.dram_tensor("x", [T, K], f32, kind="ExternalInput").ap()
    w = nc.dram_tensor("w", [O, K], f32, kind="ExternalInput").ap()
    b = nc.dram_tensor("b", [O], f32, kind="ExternalInput").ap()
    y = nc.dram_tensor("y", [T, O], f32, kind="ExternalOutput").ap()
    with tile.TileContext(nc) as tc:
        build_kernel_body_v3(tc, x, w, b, y, T, K, O, TCH=TCH)
    nc.finalize()
    return nc


_CACHED_NC = None


def _get_nc():
    global _CACHED_NC
    if _CACHED_NC is None:
        _CACHED_NC = build_bass()
    return _CACHED_NC


def make_in_maps(x, weight, bias):
    x = np.ascontiguousarray(np.asarray(x, dtype=np.float32))
    weight = np.ascontiguousarray(np.asarray(weight, dtype=np.float32))
    bias = np.ascontiguousarray(np.asarray(bias, dtype=np.float32))
    O = weight.shape[0] // N_CORES
    return [
        {
            "x": x,
            "w": weight[c * O : (c + 1) * O],
            "b": bias[c * O : (c + 1) * O],
        }
        for c in range(N_CORES)
    ]


def kernel(x, weight, bias):
    nc = _get_nc()
    in_maps = make_in_maps(x, weight, bias)
    res = run_bass_kernel_spmd(nc, in_maps, list(range(N_CORES)))
    y = np.concatenate([r["y"] for r in res.results], axis=1)  # [T, O_FULL]
    return np.ascontiguousarray(y)
